# revision 1
# baseline (speedup 1.0000x reference)
"""StyleGAN2-style modulated 3x3 conv (B=8, Ci=Co=512, H=W=32) on 8 TRN2 NeuronCores.

Sharding: data-parallel over batch, one sample per core (embarrassingly
parallel, no collectives).

Algorithm: Winograd F(2x2, 3x3). Per core:
  pad = zero-pad(x * y_s)                      [128ci, 4j, 34, 34] bf16
  RT[j,xi]   = row-combine(pad)                (B^T d)      on DVE
  V[j,xi,nu] = col-combine(RT)                 (B^T d B)    on DVE/GpSimd
  M[xi,nu]   = sum_j U[xi,nu,j]^T @ V[j,xi,nu] (PE, fp32 PSUM, N=256)
  Y1[eta,nu] = xi-combine(M)                   (A^T M)      on DVE
  Y[eta,mu]  = nu-combine(Y1)                  (A^T M A)    on DVE
  out[2ti+eta, 2tj+mu] = Y/rs + bias           scatter      on ACT
  rs = sqrt(sum_i ys2[i] w2[i,o] + eps)        demod (tiny PE matmuls)

This trades the direct conv's 288 N=512 matmuls (~62us at the bf16
roofline) for 256 N=256 matmuls (~30us) plus transforms that run on
DVE/GpSimd/ACT concurrently with the PE stream. The transformed weights
U = G w G^T (input-independent) are computed host-side and streamed as
16 co-quarter x nu slabs (512KB each) over two DMA queues in exact
consumption order; demod uses a separately shipped w2 = sum_k w_k^2.

Math note: the equal_lr scale s=(Ci*9)**-0.5 is folded out of both conv
and demod norm (eps compensated), so U/w2 come from the raw weights.
"""

import numpy as np
import ml_dtypes

import concourse.mybir as mybir
from concourse import bacc
from concourse.tile import TileContext
from concourse.bass_utils import run_bass_kernel_spmd

B = 8
CI = 512
CO = 512
H = W = 32
KK = 9
NCI = CI // 128
NCO = CO // 128
HWPAD = 34
T = 16  # winograd tile grid (16x16 tiles of 2x2 outputs)
NPT = 256  # tiles per image = T*T
XSPLIT = 17
EPS_EFF = 1e-8 * CI * KK

F32 = mybir.dt.float32
BF16 = mybir.dt.bfloat16
AF = mybir.ActivationFunctionType

# (in0_row, in1_row, op) for the B^T combine, rows taken with stride 2
# xi0 = d0-d2, xi1 = d1+d2, xi2 = d2-d1, xi3 = d1-d3
BT_COMBINE = [(0, 2, "subtract"), (1, 2, "add"), (2, 1, "subtract"), (1, 3, "subtract")]


def build_nc():
    nc = bacc.Bacc("TRN2", target_bir_lowering=False, debug=False)

    x_ext = nc.declare_dram_parameter("x", [NCI, 128, H, W], BF16, isOutput=False)
    yb_ext = nc.declare_dram_parameter("yb", [128, 2 * NCI], F32, isOutput=False)
    # transformed weights: [jo, nu, ci_p, xi, j, co_c]
    u_ext = nc.declare_dram_parameter(
        "u", [NCO, 4, 128, 4, NCI, 128], BF16, isOutput=False
    )
    # w2 = sum_k w_k^2: [ci_p, j, jo, co_c]
    w2_ext = nc.declare_dram_parameter("w2", [128, NCI, NCO, 128], BF16, isOutput=False)
    out_ext = nc.declare_dram_parameter("out", [NCO, 128, H * W], F32, isOutput=True)

    with TileContext(nc) as tc:
        with (
            tc.tile_pool(name="singles", bufs=1) as singles,
            tc.tile_pool(name="us", bufs=6) as us,
            tc.tile_pool(name="big", bufs=1) as big,
            tc.tile_pool(name="xin", bufs=1) as xin,
            tc.tile_pool(name="y1s", bufs=1) as y1s,
            tc.tile_pool(name="tmps", bufs=2) as tmps,
            tc.tile_pool(name="yts", bufs=2) as yts,
            tc.tile_pool(name="outs", bufs=2) as outs,
            tc.tile_pool(name="cps", bufs=3, space="PSUM") as cps,
            tc.tile_pool(name="dps", bufs=1, space="PSUM") as dps,
            tc.tile_pool(name="wps", bufs=1, space="PSUM") as wps,
        ):
            # ---- input DMAs, priority order per queue ----
            xt_sb = [
                xin.tile([128, H, W], BF16, tag=f"x{j}", name=f"xt{j}")
                for j in range(NCI)
            ]
            nc.gpsimd.dma_start(out=xt_sb[3], in_=x_ext[3])
            nc.sync.dma_start(out=xt_sb[0][:, 0:XSPLIT, :], in_=x_ext[0][:, 0:XSPLIT, :])
            nc.sync.dma_start(out=xt_sb[0][:, XSPLIT:H, :], in_=x_ext[0][:, XSPLIT:H, :])
            for j in range(1, NCI - 1):
                nc.sync.dma_start(out=xt_sb[j], in_=x_ext[j])

            yb_sb = singles.tile([128, 2 * NCI], F32)
            w2_sb = singles.tile([128, NCI, NCO, 128], BF16)
            u_sb = {}  # (jo, nu) -> [128, 4xi, 4j, 128co]

            def udma(engine, jo, nu, chunked=False):
                t = us.tile([128, 4, NCI, 128], BF16, tag="u", name=f"u{jo}{nu}")
                if chunked:
                    for xi in range(4):
                        engine.dma_start(out=t[:, xi], in_=u_ext[jo, nu][:, xi])
                else:
                    engine.dma_start(out=t, in_=u_ext[jo, nu])
                u_sb[(jo, nu)] = t

            # scalar queue: first slab in xi-chunks so the PE starts sooner,
            # then yb/w2, then the jo=0,2 slabs; gpsimd queue: jo=1,3 slabs.
            nc.scalar.dma_start(out=yb_sb, in_=yb_ext[:, :])
            udma(nc.scalar, 0, 0, chunked=True)
            nc.scalar.dma_start(out=w2_sb, in_=w2_ext[:, :])
            udma(nc.gpsimd, 1, 0)
            udma(nc.gpsimd, 3, 0)
            udma(nc.scalar, 2, 0)
            for nu in range(1, 4):
                udma(nc.scalar, 0, nu)
                udma(nc.gpsimd, 1, nu)
                udma(nc.scalar, 2, nu)
                udma(nc.gpsimd, 3, nu)

            # ---- PE warm-up (HAM clock gate) ----
            warm_lhs = singles.tile([128, 1], BF16)
            nc.vector.memset(warm_lhs, 1.0)
            warm_rhs = singles.tile([128, 512], BF16)
            nc.vector.memset(warm_rhs, 0.5)
            warm_ps = wps.tile([1, 512], F32)
            N_WARM = 16
            for i in range(N_WARM):
                nc.tensor.matmul(
                    out=warm_ps,
                    lhsT=warm_lhs,
                    rhs=warm_rhs,
                    start=(i == 0),
                    stop=(i == N_WARM - 1),
                )

            eps_sb = singles.tile([128, 1], F32)
            nc.vector.memset(eps_sb, EPS_EFF)

            # ---- modulated padded input + winograd input transform ----
            pad4 = big.tile([128, NCI, HWPAD, HWPAD], BF16)
            nc.gpsimd.memset(pad4[:, :, 0, :], 0.0)
            nc.gpsimd.memset(pad4[:, :, HWPAD - 1, :], 0.0)
            nc.gpsimd.memset(pad4[:, :, 1 : HWPAD - 1, 0], 0.0)
            nc.gpsimd.memset(pad4[:, :, 1 : HWPAD - 1, HWPAD - 1], 0.0)

            rt_sb = big.tile([128, 4 * NCI, T, HWPAD], BF16)  # [.., 4j+xi, ti, col]
            v_sb = big.tile([128, 16 * NCI, T, T], BF16)  # [.., 16j+4xi+nu, ti, tj]

            def mod(j, r0, r1):
                nc.vector.tensor_scalar(
                    out=pad4[:, j, 1 + r0 : 1 + r1, 1 : W + 1],
                    in0=xt_sb[j][:, r0:r1, :],
                    scalar1=yb_sb[:, j : j + 1],
                    scalar2=None,
                    op0=mybir.AluOpType.mult,
                )

            def rows(j):
                for xi, (a, b, op) in enumerate(BT_COMBINE):
                    nc.vector.tensor_tensor(
                        out=rt_sb[:, 4 * j + xi],
                        in0=pad4[:, j, a : min(a + 2 * T, HWPAD) : 2, :],
                        in1=pad4[:, j, b : min(b + 2 * T, HWPAD) : 2, :],
                        op=getattr(mybir.AluOpType, op),
                    )

            def cols(nu):
                # one op per nu over all (j, xi) at once
                a, b, op = BT_COMBINE[nu]
                nc.vector.tensor_tensor(
                    out=v_sb[:, nu : 16 * NCI : 4],
                    in0=rt_sb[:, :, :, a : min(a + 2 * T, HWPAD) : 2],
                    in1=rt_sb[:, :, :, b : min(b + 2 * T, HWPAD) : 2],
                    op=getattr(mybir.AluOpType, op),
                )

            mod(0, 0, XSPLIT)
            mod(0, XSPLIT, H)
            rows(0)
            for j in range(1, NCI):
                mod(j, 0, H)
                rows(j)
            cols(0)
            ys2_sb = singles.tile([128, NCI], BF16)
            nc.vector.tensor_mul(ys2_sb, yb_sb[:, 0:NCI], yb_sb[:, 0:NCI])
            cols(1)
            cols(2)
            cols(3)

            rs_sb = singles.tile([128, NCO], F32)
            # per-jo evacuated M (bf16): [128, 4nu, 4xi, 256]
            m_sb = [
                y1s.tile([128, 4, 4, NPT], BF16, tag=f"m_{jo}", name=f"m_{jo}")
                for jo in range(NCO)
            ]
            # per-jo Y1 (bf16): [128, 2eta, 4nu, 256]
            y1_sb = [
                y1s.tile([128, 2, 4, NPT], BF16, tag=f"y1_{jo}", name=f"y1_{jo}")
                for jo in range(NCO)
            ]
            # per-jo Y (bf16): [128, 2eta*2mu, 256]
            yt_sb = [
                y1s.tile([128, 4, NPT], BF16, tag=f"yt_{jo}", name=f"yt_{jo}")
                for jo in range(NCO)
            ]
            ot_sb = [
                outs.tile([128, H, W], F32, tag=f"ot{jo % 2}", name=f"ot{jo}")
                for jo in range(NCO)
            ]

            def unit(jo, nu):
                # M[xi] for this (jo, nu): 16 matmuls N=256, xi-outer so each
                # xi's accumulation chain is contiguous (has_written safety);
                # then one ACT op evacuates all 4 xi slices to SBUF bf16.
                ps = cps.tile([128, 4, NPT], F32, tag="ups")
                u = u_sb[(jo, nu)]
                for xi in range(4):
                    for j in range(NCI):
                        nc.tensor.matmul(
                            out=ps[:, xi],
                            lhsT=u[:, xi, j],
                            rhs=v_sb[:, 16 * j + 4 * xi + nu],
                            start=(j == 0),
                            stop=(j == NCI - 1),
                        )
                nc.scalar.activation(
                    out=m_sb[jo][:, nu], in_=ps, func=AF.Copy
                )

            TT = nc.vector.tensor_tensor
            ADD = mybir.AluOpType.add
            SUB = mybir.AluOpType.subtract

            def stage1(jo, n0, n1):
                # Y1[0,nu] = M0+M1+M2 ; Y1[1,nu] = M1-M2-M3 over nu in [n0,n1)
                m = m_sb[jo]
                y1 = y1_sb[jo]
                t = tmps.tile([128, 4, NPT], BF16, tag="t1", name="t")
                TT(out=t[:, n0:n1], in0=m[:, n0:n1, 0], in1=m[:, n0:n1, 1], op=ADD)
                TT(out=y1[:, 0, n0:n1], in0=t[:, n0:n1], in1=m[:, n0:n1, 2], op=ADD)
                TT(out=t[:, n0:n1], in0=m[:, n0:n1, 1], in1=m[:, n0:n1, 2], op=SUB)
                TT(out=y1[:, 1, n0:n1], in0=t[:, n0:n1], in1=m[:, n0:n1, 3], op=SUB)

            def stage2(jo, mu):
                # Y[:,mu0] = Y1n0+Y1n1+Y1n2 ; Y[:,mu1] = Y1n1-Y1n2-Y1n3
                y1 = y1_sb[jo]
                yt = yt_sb[jo]
                op = ADD if mu == 0 else SUB
                na, nb, ncol = (0, 1, 2) if mu == 0 else (1, 2, 3)
                t = tmps.tile([128, 2, NPT], BF16, tag="t2", name="t")
                TT(out=t, in0=y1[:, :, na], in1=y1[:, :, nb], op=op)
                TT(out=yt[:, mu : 4 : 2], in0=t, in1=y1[:, :, ncol], op=op)

            def scatter(jo, mu):
                # out[2ti+eta, 2tj+mu] = Y*rs + bias; one op per eta, ACT
                # takes eta=0 and DVE eta=1 to split the load
                yt = yt_sb[jo]
                ot = ot_sb[jo]
                nc.scalar.activation(
                    out=ot[:, 0 : H : 2, mu : W : 2],
                    in_=yt[:, mu],
                    func=AF.Identity,
                    bias=yb_sb[:, NCI + jo : NCI + jo + 1],
                    scale=rs_sb[:, jo : jo + 1],
                )
                nc.vector.tensor_scalar(
                    out=ot[:, 1 : H : 2, mu : W : 2],
                    in0=yt[:, 2 + mu],
                    scalar1=rs_sb[:, jo : jo + 1],
                    scalar2=yb_sb[:, NCI + jo : NCI + jo + 1],
                    op0=mybir.AluOpType.mult,
                    op1=mybir.AluOpType.add,
                )

            xs2_ps = dps.tile([128, NCO], F32)

            # ---- PE stream: nu-outer rounds; demod after round 0; per-jo
            # combine work staggered: nu0-2 parts after round 2, nu3 parts
            # (and the output) right after each round-3 unit ----
            for jo in range(NCO):
                unit(jo, 0)
            for jo in range(NCO):
                for j in range(NCI):
                    nc.tensor.matmul(
                        out=xs2_ps[:, jo : jo + 1],
                        lhsT=w2_sb[:, j, jo],
                        rhs=ys2_sb[:, j : j + 1],
                        start=(j == 0),
                        stop=(j == NCI - 1),
                    )
            nc.scalar.activation(out=rs_sb, in_=xs2_ps, func=AF.Sqrt, bias=eps_sb)
            nc.vector.reciprocal(out=rs_sb, in_=rs_sb)
            for jo in range(NCO):
                unit(jo, 1)
            for jo in range(NCO):
                unit(jo, 2)
                stage1(jo, 0, 3)
                stage2(jo, 0)
                scatter(jo, 0)
            for jo in range(NCO):
                unit(jo, 3)
                stage1(jo, 3, 4)
                stage2(jo, 1)
                scatter(jo, 1)
                nc.sync.dma_start(out=out_ext[jo], in_=ot_sb[jo])

            warm_sink = singles.tile([1, 1], F32)
            nc.vector.tensor_copy(out=warm_sink, in_=warm_ps[0:1, 0:1])
    nc.compile()
    return nc


_NC_CACHE = None


def _get_nc():
    global _NC_CACHE
    if _NC_CACHE is None:
        _NC_CACHE = build_nc()
    return _NC_CACHE


_G = np.array(
    [[1, 0, 0], [0.5, 0.5, 0.5], [0.5, -0.5, 0.5], [0, 0, 1]], np.float64
)


def _prep_inputs(x, y_s, weight, bias):
    w = weight.astype(np.float64)
    # U[xi, nu, co, ci] = G w G^T (input-independent weight transform)
    u = np.einsum("xa,nb,oiab->xnoi", _G, _G, w)
    # arrange to [jo, nu, ci_p, xi, j, co_c]
    u6 = u.reshape(4, 4, NCO, 128, NCI, 128).transpose(2, 1, 5, 0, 4, 3)
    u_arr = np.ascontiguousarray(u6).astype(ml_dtypes.bfloat16)
    w2 = (w**2).sum(axis=(2, 3))  # [co, ci]
    # [ci_p, j, jo, co_c]
    w2_arr = np.ascontiguousarray(
        w2.reshape(NCO, 128, NCI, 128).transpose(3, 2, 0, 1)
    ).astype(ml_dtypes.bfloat16)
    in_maps = []
    for b in range(B):
        yb = np.empty((128, 2 * NCI), np.float32)
        yb[:, :NCI] = y_s[b].reshape(NCI, 128).T
        yb[:, NCI:] = bias.reshape(NCO, 128).T
        in_maps.append(
            {
                "x": np.ascontiguousarray(x[b].reshape(NCI, 128, H, W)).astype(
                    ml_dtypes.bfloat16
                ),
                "yb": yb,
                "u": u_arr,
                "w2": w2_arr,
            }
        )
    return in_maps


def _install_trace_support():
    """Dev-only: register the axon NTFF profiling hook + disable the
    remote artifact upload so trace=True works in this container."""
    import sys
    import types

    import concourse.bass_utils as bu

    bu.upload_artifacts = lambda tmpdir: "local://" + str(tmpdir)
    if "antenv.axon_hooks" in sys.modules:
        return
    try:
        from trn_agent_boot.trn_boot import _ntff_profile_via_ctypes

        hook = _ntff_profile_via_ctypes("/opt/axon/libaxon_pjrt.so")
    except Exception:
        return
    mod = types.ModuleType("antenv.axon_hooks")
    mod.get_axon_ntff_profile_hook = lambda: hook
    mod.set_axon_ntff_profile_hook = lambda h: None
    sys.modules["antenv.axon_hooks"] = mod


def run(x, y_s, weight, bias, trace=False, tmpdir=None):
    nc = _get_nc()
    if trace:
        _install_trace_support()
    in_maps = _prep_inputs(x, y_s, weight, bias)
    res = run_bass_kernel_spmd(
        nc, in_maps, core_ids=list(range(B)), trace=trace, tmpdir=tmpdir
    )
    out = np.stack(
        [res.results[b]["out"].reshape(CO, H, W) for b in range(B)]
    ).astype(np.float32)
    return out, res


def kernel(x, y_s, weight, bias):
    out, _ = run(
        np.asarray(x, dtype=np.float32),
        np.asarray(y_s, dtype=np.float32),
        np.asarray(weight, dtype=np.float32),
        np.asarray(bias, dtype=np.float32),
    )
    return out

